# revision 54
# baseline (speedup 1.0000x reference)
"""Trainium2 Bass kernel: causal self-attention with RoPE (nn_Attention_71339406786815).

Full inputs -> full output. Internally shards across 8 NeuronCores:
  core c: batch b = c//4, head-group g = c%4 (4 heads x 128 dims = 512 features).
Each core computes q/k/v projections for its head group, RoPE, causal
attention, and the row-parallel slice of the output projection; the host
sums the 4 partial outputs per batch (standard tensor-parallel reduction).
No collectives: every core's work is independent.

v2 design (vs the f32r baseline):
  * everything bf16 on the PE; weights/x/cos/sin pre-cast to bf16 on host.
  * weights resident in SBUF (loaded once, 8 MB) instead of re-DMAed per
    chunk (was 96 MB of HBM traffic per core).
  * x^T materialized by DMA-transpose (XBAR) instead of PE transposes,
    freeing PE cycles, PSUM banks and the DVE evacuation copies.
  * phase B computes scores TRANSPOSED (S^T[k,q] = kT-block^T @ qT) so the
    probabilities come out of the exp already in the [k, q] layout the
    ctx matmul needs -- no per-block PE transpose of the probabilities.
    The softmax denominator (a k-sum = partition-dim sum) is accumulated
    by the otherwise-idle Pool engine (tensor adds + partition_all_reduce)
    and folded into the PSUM->SBUF evacuation of ctx^T as a reciprocal
    multiply (DVE).  Scores are trimmed to the causal range; the dead
    region of each prob tile is memset to 0 so the full-width ctx matmul
    reads zeros.

Layouts (per core):
  qT/kT: [128, 4, T] bf16 -- tile h = head h, partition = head dim, free = t
  vN:    [128, 16, 512] bf16 -- natural [t%128, t//128, e]
  cT:    [128, 4, T] bf16 -- ctx^T (normalized)
"""

import math
import sys

import numpy as np

sys.path.insert(0, "/opt/trn_rl_repo")

T = 2048          # sequence length
D = 2048          # d_model
B = 2             # batch
E = 512           # features per head-group (4 heads x 128)
DH = 128          # head dim
HEADS_PER_CORE = 4
N_CORES = 8
SCALE = 1.0 / math.sqrt(DH)
ROPE_BASE = 10000.0
NEG_INF = -1e30
CH = 512          # phase-A token chunk
QC = 512          # phase-B query chunk

_CACHE = {}


def _build(seq=T, dump=False):
    """Build + compile the per-core Bass program (SPMD: same program, 8 cores)."""
    import concourse.mybir as mybir
    import concourse.tile as tile
    from concourse import bacc

    f32 = mybir.dt.float32
    bf16 = mybir.dt.bfloat16
    Exp = mybir.ActivationFunctionType.Exp

    n_ch = seq // CH        # phase-A chunks
    n_dt = D // 128         # 16 contraction tiles
    n_qb = seq // 128       # 128-token blocks
    n_qc = seq // QC        # phase-B query chunks
    qb_per_qc = QC // 128   # 4
    n_et = HEADS_PER_CORE

    nc = bacc.Bacc(None, target_bir_lowering=False, debug=False)

    x_d = nc.declare_dram_parameter("xT", [D, seq], bf16, isOutput=False)
    wq_d = nc.declare_dram_parameter("wq", [D, E], bf16, isOutput=False)
    wk_d = nc.declare_dram_parameter("wk", [D, E], bf16, isOutput=False)
    wv_d = nc.declare_dram_parameter("wv", [D, E], bf16, isOutput=False)
    wo_d = nc.declare_dram_parameter("wo", [E, D], bf16, isOutput=False)
    cos_d = nc.declare_dram_parameter("cosf", [128, seq], bf16, isOutput=False)
    sin_d = nc.declare_dram_parameter("sinf", [128, seq], bf16, isOutput=False)
    cm_d = nc.declare_dram_parameter("cmaskT", [128, 128], f32, isOutput=False)
    out_d = nc.declare_dram_parameter("out", [seq, D], bf16, isOutput=True)
    if dump:
        dmp = {
            "d_qT": nc.declare_dram_parameter("d_qT", [128, n_et, seq], bf16, isOutput=True),
            "d_kT": nc.declare_dram_parameter("d_kT", [128, n_et, seq], bf16, isOutput=True),
            "d_cT": nc.declare_dram_parameter("d_cT", [128, n_et, seq], bf16, isOutput=True),
            "d_vN": nc.declare_dram_parameter("d_vN", [128, seq // 128, E], bf16, isOutput=True),
        }

    xv = x_d[:].rearrange("(dt p) t -> p dt t", p=128)            # [128,16,seq]
    wqv = wq_d[:].rearrange("(k p) e -> p k e", p=128)            # [128,16,E]
    wkv = wk_d[:].rearrange("(k p) e -> p k e", p=128)
    wvv = wv_d[:].rearrange("(k p) e -> p k e", p=128)
    wov = wo_d[:].rearrange("(et p) n -> p et n", p=128)          # [128,4,D]
    ov = out_d[:].rearrange("(tt p) n -> tt p n", p=128)          # [n_qb,128,D]

    with tile.TileContext(nc) as tc:
        with (
            tc.tile_pool(name="consts", bufs=1) as consts,
            tc.tile_pool(name="weights", bufs=1) as wpool,
            tc.tile_pool(name="persist", bufs=1) as persist,
        ):
            # [128,1] bf16 ones: stationary for the PE softmax-denominator
            ones_sb = consts.tile([128, 1], bf16)
            nc.vector.memset(ones_sb[:], 1.0)

            # Weight/const loads ride the scalar (Act) HWDGE queue, ordered
            # by first use (wq/wk before cos/sin before wv before wo) so
            # the first projection matmuls start as early as possible.
            # Chunk-0 x loads are hoisted ahead of everything (see below).
            cos_sb = consts.tile([128, seq], bf16)
            sin_sb = consts.tile([128, seq], bf16)
            cmT_sb = consts.tile([128, 128], f32)
            wq_sb = wpool.tile([128, n_dt, E], bf16)
            wk_sb = wpool.tile([128, n_dt, E], bf16)
            wv_sb = wpool.tile([128, n_dt, E], bf16)
            wo_sb = wpool.tile([128, n_et, D], bf16)

            def load_weights_early():
                # wq on the scalar queue, wk on the sync queue (behind the
                # respective halves of chunk-0's x loads): each projection's
                # weight quarters land just-in-time for its matmul stream,
                # and chunk-1's x loads queue up right behind.
                for i in range(0, n_dt, 4):
                    nc.sync.dma_start(wq_sb[:, i : i + 4, :], wqv[:, i : i + 4, :])
                for i in range(0, n_dt, 4):
                    nc.sync.dma_start(wk_sb[:, i : i + 4, :], wkv[:, i : i + 4, :])
                nc.scalar.dma_start(cos_sb[:], cos_d[:])
                nc.scalar.dma_start(sin_sb[:], sin_d[:])

            def load_weights_late():
                for i in range(0, n_dt, 4):
                    nc.sync.dma_start(wv_sb[:, i : i + 4, :], wvv[:, i : i + 4, :])
                nc.scalar.dma_start(cmT_sb[:], cm_d[:])
                for i in range(n_et):
                    nc.sync.dma_start(wo_sb[:, i, :], wov[:, i, :])

            qT = persist.tile([128, n_et, seq], bf16)   # [dh, head, t]
            kT = persist.tile([128, n_et, seq], bf16)
            vN = persist.tile([128, n_qb, E], bf16)     # [t%128, t//128, e]
            cT = persist.tile([128, n_et, seq], bf16)   # ctx^T, normalized

            # ---------------- Phase A: x^T (DMA xbar), projections, RoPE ----
            with (
                tc.tile_pool(name="xt", bufs=2) as xtp,
                tc.tile_pool(name="ra", bufs=8) as rap,
                tc.tile_pool(name="psa", bufs=8, space="PSUM") as psap,
            ):
                # x arrives host-pre-transposed; straight strided loads at
                # full DMA rate, alternating the two HWDGE queues.  Chunk 0
                # is dispatched before the weight loads.
                def load_chunk(c):
                    xtc = xtp.tile([128, n_dt, CH], bf16, tag="xt")
                    cs = slice(c * CH, (c + 1) * CH)
                    for dt in range(n_dt):
                        nc.scalar.dma_start(xtc[:, dt, :], xv[:, dt, cs])
                    return xtc

                xtc_next = load_chunk(0)
                load_weights_early()

                for c in range(n_ch):
                    ts_ = slice(c * CH, (c + 1) * CH)
                    xtc = xtc_next
                    if c + 1 < n_ch:
                        xtc_next = load_chunk(c + 1)
                    if c == 0:
                        load_weights_late()
                    # q/k projections + RoPE
                    for wsb, dst in ((wq_sb, qT), (wk_sb, kT)):
                        pp = [
                            psap.tile([128, CH], f32, tag="psa", name=f"pp{i}")
                            for i in range(n_et)
                        ]
                        for dt in range(n_dt):
                            for et in range(n_et):
                                nc.tensor.matmul(
                                    pp[et][:],
                                    wsb[:, dt, et * 128 : (et + 1) * 128],
                                    xtc[:, dt, :],
                                    start=(dt == 0), stop=(dt == n_dt - 1),
                                )
                        # RoPE: dst = raw*cos + swap(raw)*sin_signed; the
                        # partition swap (p <-> p^64) uses SBUF->SBUF DMAs
                        # dispatched from the (otherwise idle) gpsimd queue
                        # so the sync queue stays clear for x transposes.
                        for et in range(n_et):
                            raw = rap.tile([128, CH], bf16, tag="raw")
                            nc.scalar.copy(raw[:], pp[et][:])
                            sw = rap.tile([128, CH], bf16, tag="sw")
                            nc.gpsimd.dma_start(sw[0:64, :], raw[64:128, :])
                            nc.gpsimd.dma_start(sw[64:128, :], raw[0:64, :])
                            m1 = rap.tile([128, CH], bf16, tag="m1")
                            nc.vector.tensor_mul(m1[:], raw[:], cos_sb[:, ts_])
                            m2 = rap.tile([128, CH], bf16, tag="m2")
                            nc.vector.tensor_mul(m2[:], sw[:], sin_sb[:, ts_])
                            nc.vector.tensor_add(dst[:, et, ts_], m1[:], m2[:])
                    # v projection (natural [t, e] layout)
                    pv = [
                        psap.tile([128, E], f32, tag="psa", name=f"pv{i}")
                        for i in range(CH // 128)
                    ]
                    for dt in range(n_dt):
                        for s in range(CH // 128):
                            nc.tensor.matmul(
                                pv[s][:],
                                xtc[:, dt, s * 128 : (s + 1) * 128],
                                wv_sb[:, dt, :],
                                start=(dt == 0), stop=(dt == n_dt - 1),
                            )
                    for s in range(CH // 128):
                        nc.scalar.copy(vN[:, c * (CH // 128) + s, :], pv[s][:])

            # ------- Phase B: transposed-score causal attention + out-proj ---
            # Per (qc, h) the kb loop is software-pipelined: the score
            # matmul for kb+1 is emitted BEFORE the exp-dependent ones/ctx
            # matmuls of kb, so the in-order PE queue streams scores while
            # the Act engine runs the exp.  The softmax denominator is a
            # [1, QC] PSUM accumulator fed by a ones-stationary matmul
            # (rides the PE queue -- no cross-engine serial chain).
            with (
                tc.tile_pool(name="prb", bufs=4) as prp,
                tc.tile_pool(name="rr1p", bufs=2) as rr1p,
                tc.tile_pool(name="rrp", bufs=2) as rrp,
                tc.tile_pool(name="ob", bufs=3) as obp,
                tc.tile_pool(name="psc", bufs=2, space="PSUM") as pscp,
                tc.tile_pool(name="pcx", bufs=2, space="PSUM") as pcxp,
                tc.tile_pool(name="psm", bufs=2, space="PSUM") as psmp,
                tc.tile_pool(name="pso", bufs=2, space="PSUM") as psop,
            ):
                def emit_outproj(tt, tail=False):
                    for nk in range(4):
                        po = psop.tile([128, 512], f32, tag="po")
                        for et in range(n_et):
                            nc.tensor.matmul(
                                po[:],
                                cT[:, et, tt * 128 : (tt + 1) * 128],
                                wo_sb[:, et, nk * 512 : (nk + 1) * 512],
                                start=(et == 0), stop=(et == n_et - 1),
                            )
                        ob = obp.tile([128, 512], bf16, tag="ob")
                        if tail:
                            # final flush: Act is idle (no exps left), DVE
                            # still owns the last heads' normalize chain
                            nc.scalar.copy(ob[:], po[:])
                        else:
                            nc.vector.tensor_copy(ob[:], po[:])
                        nc.sync.dma_start(
                            ov[tt][:, nk * 512 : (nk + 1) * 512], ob[:]
                        )

                for qc in range(n_qc):
                    q0 = qc * QC
                    nkb = qb_per_qc * (qc + 1)

                    def d0_of(kb):
                        return max(0, (kb - qb_per_qc * qc) * 128)

                    # heads processed in interleaved pairs: two independent
                    # score->exp->consume streams keep the in-order PE queue
                    # fed while either head's exp is in flight.
                    for hp in (0, 2):
                        st = {
                            h: {
                                "cx": pcxp.tile([128, QC], f32, tag="cx",
                                                name=f"cx{h}"),
                                "sums": psmp.tile([1, QC], f32, tag="sums",
                                                  name=f"sm{h}"),
                                "prs": {},
                            }
                            for h in (hp, hp + 1)
                        }

                        def emit_score(h, kb):
                            d0 = d0_of(kb)
                            sc = pscp.tile([128, QC], f32, tag="sc")
                            nc.tensor.matmul(
                                sc[:, d0:QC],
                                kT[:, h, kb * 128 : (kb + 1) * 128],
                                qT[:, h, q0 + d0 : q0 + QC],
                                start=True, stop=True,
                            )
                            if kb >= qb_per_qc * qc:  # diagonal: mask
                                nc.vector.tensor_add(
                                    sc[:, d0 : d0 + 128],
                                    sc[:, d0 : d0 + 128],
                                    cmT_sb[:],
                                )
                            pr = prp.tile([128, QC], bf16, tag="pr")
                            nc.scalar.activation(
                                pr[:, d0:QC], sc[:, d0:QC], Exp,
                                bias=0.0, scale=SCALE,
                            )
                            st[h]["prs"][kb] = pr

                        def emit_consume(h, kb):
                            # consumers trimmed to the causal range: the
                            # region left of d0 is never touched (kb==0
                            # covers the full width, so the accumulators
                            # are fully initialized).  The k-sum partials
                            # accumulate on DVE/Pool (split by head parity)
                            # instead of burning PE issue slots.
                            d0 = d0_of(kb)
                            pr = st[h]["prs"].pop(kb)
                            nc.tensor.matmul(
                                st[h]["sums"][:, d0:QC],
                                ones_sb[:], pr[:, d0:QC],
                                start=(kb == 0), stop=(kb == nkb - 1),
                            )
                            nc.tensor.matmul(
                                st[h]["cx"][:, d0:QC],
                                vN[:, kb, h * 128 : (h + 1) * 128],
                                pr[:, d0:QC],
                                start=(kb == 0), stop=(kb == nkb - 1),
                            )

                        for kb in range(nkb + 1):
                            for h in (hp, hp + 1):
                                if kb < nkb:
                                    emit_score(h, kb)
                                if kb >= 1:
                                    emit_consume(h, kb - 1)

                        for h in (hp, hp + 1):
                            rr1 = rr1p.tile([1, QC], f32, tag="rr1")
                            nc.vector.reciprocal_approx_fast(
                                rr1[:], st[h]["sums"][:]
                            )
                            rr = rrp.tile([128, QC], f32, tag="rr")
                            nc.gpsimd.partition_broadcast(rr[:], rr1[:])
                            nc.vector.tensor_mul(
                                cT[:, h, q0 : q0 + QC], st[h]["cx"][:], rr[:]
                            )
                            # previous query chunk's out-projection, spread
                            # one tt-block per head: dense PE filler while
                            # the Act engine drains the exp backlog.
                            if qc > 0:
                                emit_outproj(qb_per_qc * (qc - 1) + h)
                for tt in range(qb_per_qc * (n_qc - 1), qb_per_qc * n_qc):
                    emit_outproj(tt, tail=True)

            if dump:
                for ndst, tsrc in (("d_qT", qT), ("d_kT", kT), ("d_vN", vN),
                                   ("d_cT", cT)):
                    nc.sync.dma_start(dmp[ndst][:], tsrc[:])

    nc.compile()
    return nc


def _prep_in_maps(x, q_out, k_out, v_out, w_out, pos, seq=T):
    import ml_dtypes

    bf16 = ml_dtypes.bfloat16
    x = np.asarray(x, dtype=np.float32)
    q_out = np.asarray(q_out, dtype=np.float32)
    k_out = np.asarray(k_out, dtype=np.float32)
    v_out = np.asarray(v_out, dtype=np.float32)
    w_out = np.asarray(w_out, dtype=np.float32)
    start = max(int(np.asarray(pos)), 0)

    half = DH // 2  # 64
    inv = 1.0 / (ROPE_BASE ** (np.arange(0, DH, 2, dtype=np.float64) / DH))  # [64]
    tpos = np.arange(start, start + seq, dtype=np.float64)
    ang = tpos[:, None] * inv[None, :]                     # [seq, 64]
    cosf = np.cos(ang).T.astype(np.float32)                # [64, seq]
    sinf = np.sin(ang).T.astype(np.float32)
    cos128 = np.ascontiguousarray(np.tile(cosf, (128 // half, 1))).astype(bf16)
    sgn = np.where((np.arange(128) % DH) < half, -1.0, 1.0).astype(np.float32)
    sin128 = np.ascontiguousarray(
        np.tile(sinf, (128 // half, 1)) * sgn[:, None]
    ).astype(bf16)
    # transposed causal mask: partition = k (within block), free = q
    cmaskT = np.where(
        np.arange(128)[:, None] > np.arange(128)[None, :], NEG_INF, 0.0
    ).astype(np.float32)

    in_maps = []
    for c in range(N_CORES):
        b, g = c // 4, c % 4
        F = slice(g * E, (g + 1) * E)
        in_maps.append({
            "xT": np.ascontiguousarray(x[b, :seq].T).astype(bf16),
            "wq": np.ascontiguousarray(q_out[:, F]).astype(bf16),
            "wk": np.ascontiguousarray(k_out[:, F]).astype(bf16),
            "wv": np.ascontiguousarray(v_out[:, F]).astype(bf16),
            "wo": np.ascontiguousarray(w_out[F, :]).astype(bf16),
            "cosf": cos128,
            "sinf": sin128,
            "cmaskT": cmaskT,
        })
    return in_maps


def _run(in_maps, seq=T, dump=False, **kw):
    from concourse.bass_utils import run_bass_kernel_spmd

    key = ("nc", seq, dump)
    if key not in _CACHE:
        _CACHE[key] = _build(seq, dump=dump)
    return run_bass_kernel_spmd(_CACHE[key], in_maps, core_ids=list(range(N_CORES)), **kw)


def kernel(x, q_out, k_out, v_out, w_out, pos):
    in_maps = _prep_in_maps(x, q_out, k_out, v_out, w_out, pos)
    res = _run(in_maps).results
    out = np.empty((B, T, D), dtype=np.float32)
    for b in range(B):
        out[b] = (
            res[4 * b + 0]["out"].astype(np.float32)
            + res[4 * b + 1]["out"].astype(np.float32)
            + res[4 * b + 2]["out"].astype(np.float32)
            + res[4 * b + 3]["out"].astype(np.float32)
        )
    return out
